# revision 24
# baseline (speedup 1.0000x reference)
"""Trainium2 Bass kernel for nn_BiAttention.

Data-parallel over batch across 8 NeuronCores (2 batches per core).

Per batch (QL=512, CL=2048, D=2048), with S[i,j] = sc[i] + sq[j] + G[i,j],
G = (c*wm) @ q^T:
  - alpha = softmax_j(S): the sc[i] term cancels in the row softmax, so we
    exponentiate E[j,i] = exp(G^T[j,i] + sq[j]) directly (values are O(1),
    no max subtraction needed) and normalize lazily via r[i] = 1/sum_j E.
  - context2question = (E^T @ q) * r  (normalization folded into the
    PSUM-evacuating multiply with c).
  - beta = softmax_i(max_j S) = softmax_i(sc[i] + log max_j E') ->
    b[i] = (max_j E[j,i]) * exp(sc[i]), q2c = (b @ c) / sum(b).

Layouts: everything that feeds the d-contraction (matmul 1) is transposed
on-chip with PE transpose-mode matmuls (fp32 has no DMA transpose; PE
transposes are ~50-110ns per 128x128 tile).  All matmuls run in bf16
(1 cycle/row on TRN2 vs 4 for fp32); accumulation is fp32 in PSUM.
The small wm scale makes the bf16 S error ~1.5e-3 absolute, measured
overall rel-l2 error ~3e-4.
"""

from contextlib import ExitStack

import numpy as np

import concourse.bass as bass
import concourse.mybir as mybir
import concourse.tile as tile
from concourse import bacc
from concourse.bass import ts
from concourse.bass_utils import run_bass_kernel_spmd
from concourse.masks import make_identity

P = 128
B, QL, CL, D = 16, 512, 2048, 2048
N_CORES = 8
B_LOC = B // N_CORES

F32 = mybir.dt.float32
BF16 = mybir.dt.bfloat16
F8 = mybir.dt.float8e4
AX = mybir.AxisListType
ALU = mybir.AluOpType
ACTF = mybir.ActivationFunctionType
DROW = mybir.MatmulPerfMode.DoubleRow
# fp8 prescale for the wm/wc-scaled operands (q*wm ~ N(0, 0.013) would land
# in e4m3's subnormal range; x64 centers it). Un-done in the exp scale.
FP8_SCALE = 64.0


CFG = {
    "io_bufs": 2,
    "outp_bufs": 2,
    "e_bufs": 2,
    "work_bufs": 2,
    "psb_bufs": 2,
    "o3_dve_mod": 2,        # ig % mod != 0 -> DVE, else GPSIMD
    "ct_act_mod": 2,        # d % mod != 0 -> ACT, else DVE for cT evac
    "ct_bufs": 2,
}


def emit(ctx, nc, tc, q_in, c_in, w_in, out, B_loc, QLd, CLd, Dd, rep=1):
    """Emit the Tile program. Dimensions parameterized for small-shape sim.

    rep>1 repeats the whole body (idempotent writes) for marginal-time
    benchmarking; the harness path always uses rep=1."""
    JC = QLd // P          # j-chunks (4)
    DC = Dd // P           # d-chunks (16)
    IC = CLd // P          # i-chunks (16)
    IBW = min(512, CLd)    # i-block width
    IB = IBW // P          # i-chunks per block (4)
    NBLK = IC // IB        # i-blocks per batch (4)
    DBW = min(512, Dd)     # d-block width for M2/q2c/outputs
    DB = Dd // DBW         # d-blocks (4)

    const = ctx.enter_context(tc.tile_pool(name="const", bufs=1))
    io = ctx.enter_context(tc.tile_pool(name="io", bufs=CFG["io_bufs"]))
    res = ctx.enter_context(tc.tile_pool(name="res", bufs=1))
    ctp = ctx.enter_context(tc.tile_pool(name="ctp", bufs=1))
    work = ctx.enter_context(tc.tile_pool(name="work", bufs=CFG["work_bufs"]))
    work1 = ctx.enter_context(tc.tile_pool(name="work1", bufs=1))
    outp = ctx.enter_context(tc.tile_pool(name="outp", bufs=CFG["outp_bufs"]))
    small = ctx.enter_context(tc.tile_pool(name="small", bufs=2))
    rows = ctx.enter_context(tc.tile_pool(name="rows", bufs=1))
    dram = ctx.enter_context(tc.tile_pool(name="dram", bufs=2, space="DRAM"))
    ps_big = ctx.enter_context(tc.tile_pool(name="psb", bufs=CFG["psb_bufs"], space="PSUM"))
    ps_min = ctx.enter_context(tc.tile_pool(name="psm", bufs=1, space="PSUM"))

    # ---- constants ----
    wcol = const.tile([P, 3 * DC], F32)      # (p, col) = wsim[col*128 + p]
    nc.sync.dma_start(wcol, w_in.rearrange("(o p) -> p o", p=P))
    wcol_b = const.tile([P, 3 * DC], BF16)
    nc.vector.tensor_copy(wcol_b, wcol)
    wqf = io.tile([P, Dd], F32, tag="fio", name="wqf")
    nc.sync.dma_start(wqf, w_in[None, 0:Dd].to_broadcast([P, Dd]))
    wq_bc = const.tile([P, Dd], BF16)        # wq broadcast to all partitions
    nc.vector.tensor_copy(wq_bc, wqf)
    ident = const.tile([P, P], BF16)
    make_identity(nc, ident)
    ones_col = const.tile([P, 1], BF16)
    nc.vector.memset(ones_col, 1.0)
    ones_col_f = const.tile([P, 1], F32)
    nc.vector.memset(ones_col_f, 1.0)
    ones_11 = const.tile([1, 1], BF16)
    nc.vector.memset(ones_11, 1.0)
    # x64-prescaled wm (f32, per-partition scale for qmT evac) and wc (fp8
    # matmul operand for the sc row)
    wm64 = const.tile([P, DC], F32)
    nc.vector.tensor_scalar(wm64, wcol[:, 2 * DC:3 * DC], FP8_SCALE, None,
                            ALU.mult)
    wc64_8 = const.tile([P, DC], F8)
    nc.vector.tensor_scalar(wc64_8, wcol[:, DC:2 * DC], FP8_SCALE, None,
                            ALU.mult)

    def q_phase(b):
        # Q phase: load q, cast, sq, transposed+wm-scaled qmT
        qb = []
        for j in range(JC):
            qf = io.tile([P, Dd], F32, tag="fio", name="qf")
            nc.sync.dma_start(qf, q_in[b, ts(j, P), :])
            t = res.tile([P, Dd], BF16, tag=f"qb{j}", name=f"qb{j}")
            nc.vector.tensor_copy(t, qf)
            qb.append(t)
        sq = []
        for j in range(JC):
            scr = work1.tile([P, Dd], BF16, tag="ttr_scr", name="ttr_scr")
            s = small.tile([P, 1], F32, tag=f"sq{j}", name=f"sq{j}")
            nc.vector.tensor_mul(scr, qb[j], wq_bc)
            nc.vector.reduce_sum(s, scr, axis=AX.X)
            sq.append(s)
        # qmT8[t][:, k, :] = (q^T * wm * 64)[d-chunk 2t+k] in fp8, pairs
        # packed in the free dim for DoubleRow matmuls.
        qmT = []
        for t2 in range(DC // 2):
            pair = res.tile([P, 2, QLd], F8, tag=f"qmT{t2}", name=f"qmT{t2}")
            for k in range(2):
                d = 2 * t2 + k
                tp = ps_big.tile([P, QLd], BF16, tag="tp", name="tp",
                                 bufs=CFG.get("tp_bufs", 2))
                for j in range(JC):
                    nc.tensor.transpose(tp[:, ts(j, P)], qb[j][:, ts(d, P)],
                                        ident)
                # evacuate PSUM with per-partition (=per-d) wm*64 scale folded
                # in, casting to fp8
                nc.scalar.activation(pair[:, k, :], tp, ACTF.Copy, bias=0.0,
                                     scale=wm64[:, d:d + 1])
            qmT.append(pair)
        return qb, sq, qmT

    for it in range(rep * B_loc):
        b = it % B_loc
        qb, sq, qmT = q_phase(b)

        b_all = small.tile([P, IC], BF16, tag="ball", name="ball")
        q2c_acc = rows.tile([1, Dd], F32, tag="q2ca", name="q2ca")
        cb = [None] * IC

        for blk in range(NBLK):
            # ---- load c rows directly as bf16 (SWDGE cast-DMA), write O1
            # back as f32 (SWDGE cast). O1 is bf16-rounded: ~6e-4 rel_l2. ----
            for ii in range(IB):
                ig = blk * IB + ii
                t = res.tile([P, Dd], BF16, tag=f"cb{ig}", name=f"cb{ig}")
                nc.gpsimd.dma_start(t, c_in[b, ts(ig, P), :])
                nc.gpsimd.dma_start(out[b, ts(ig, P), 0:Dd], t)  # O1 = c
                cb[ig] = t
            # ---- transpose block of c -> fp8 pairs for DoubleRow ----
            cT = []
            for t2 in range(DC // 2):
                pair = ctp.tile([P, 2, IBW], F8, tag=f"ct{t2}", name=f"ct{t2}",
                                bufs=CFG.get("ct_bufs", 1))
                for k in range(2):
                    d = 2 * t2 + k
                    tp = ps_big.tile([P, IBW], BF16, tag="tp", name="tp",
                                     bufs=CFG.get("tp_bufs", 2))
                    for ii in range(IB):
                        nc.tensor.transpose(tp[:, ts(ii, P)],
                                            cb[blk * IB + ii][:, ts(d, P)],
                                            ident)
                    if d % CFG["ct_act_mod"] == 0:
                        nc.vector.tensor_copy(pair[:, k, :], tp)
                    else:
                        nc.scalar.copy(pair[:, k, :], tp)
                cT.append(pair)
            # ---- M1: S^T*64 = (qm*64)^T^T @ cT via fp8 DoubleRow, then
            # exp with the 1/64 un-scale folded into the activation ----
            E = []
            for j in range(JC):
                m1 = ps_big.tile([P, IBW], F32, tag="m1", name="m1")
                for t2 in range(DC // 2):
                    nc.tensor.matmul(m1, lhsT=qmT[t2][:, :, ts(j, P)],
                                     rhs=cT[t2], perf_mode=DROW,
                                     start=(t2 == 0), stop=(t2 == DC // 2 - 1))
                e = work.tile([P, IBW], BF16, tag=f"e{j}", name=f"e{j}",
                              bufs=CFG["e_bufs"])
                nc.scalar.activation(e, m1, ACTF.Exp, bias=sq[j],
                                     scale=1.0 / FP8_SCALE)
                E.append(e)
            # ---- Z row -> r = 1/Z, broadcast to all partitions via DRAM ----
            z = ps_min.tile([1, IBW], F32, tag="mini", name="mini",
                            bufs=CFG.get("mini_bufs", 1))
            for j in range(JC):
                nc.tensor.matmul(z, lhsT=ones_col, rhs=E[j],
                                 start=(j == 0), stop=(j == JC - 1))
            r_row = small.tile([1, IBW], F32, tag="rrow", name="rrow")
            nc.vector.reciprocal(r_row, z)
            r_rowb = small.tile([1, IBW], BF16, tag="rrowb", name="rrowb")
            nc.vector.tensor_copy(r_rowb, r_row)
            r_bc = work.tile([P, IBW], BF16, tag="rbc", name="rbc")
            nc.gpsimd.partition_broadcast(r_bc, r_rowb)
            # ---- max over j (partition dim, unnormalized E) ----
            mx = work.tile([P, IBW], BF16, tag="mx", name="mx")
            nc.vector.tensor_copy(mx, E[0])
            for j in range(1, JC):
                nc.vector.tensor_max(mx, mx, E[j])
            # ---- normalize E in place: alpha^T = E * r (r per free-position) ----
            for j in range(JC):
                nc.vector.tensor_mul(E[j], E[j], r_bc)
            # ---- sc row for this block (x64-scaled, fp8 operands) ----
            scp = ps_min.tile([1, IBW], F32, tag="mini", name="mini", bufs=CFG.get("mini_bufs", 1))
            for d in range(DC):
                nc.tensor.matmul(scp, lhsT=wc64_8[:, d:d + 1],
                                 rhs=cT[d // 2][:, d % 2, :],
                                 start=(d == 0), stop=(d == DC - 1))
            scrow = small.tile([1, IBW], BF16, tag="scrow", name="scrow")
            nc.scalar.copy(scrow, scp)
            for ii in range(IB):
                ig = blk * IB + ii
                mt = ps_min.tile([P, P], BF16, tag="minib", name="minib")
                nc.tensor.transpose(mt, mx[:, ts(ii, P)], ident)
                mcol = small.tile([P, 1], F32, tag="mcol", name="mcol")
                nc.vector.reduce_max(mcol, mt, axis=AX.X)
                scc = ps_min.tile([P, 1], F32, tag="mini", name="mini", bufs=CFG.get("mini_bufs", 1))
                nc.tensor.matmul(scc, lhsT=scrow[0:1, ts(ii, P)], rhs=ones_11)
                esc = small.tile([P, 1], F32, tag="esc", name="esc")
                nc.scalar.activation(esc, scc, ACTF.Exp,
                                     scale=1.0 / FP8_SCALE)
                nc.vector.tensor_mul(b_all[:, ig:ig + 1], mcol, esc)
            # ---- q2c partial sums (contract i within block) ----
            for db in range(DB):
                qp = ps_min.tile([1, DBW], F32, tag="mini", name="mini", bufs=CFG.get("mini_bufs", 1))
                for ii in range(IB):
                    ig = blk * IB + ii
                    nc.tensor.matmul(qp, lhsT=b_all[:, ig:ig + 1],
                                     rhs=cb[ig][:, ts(db, DBW)],
                                     start=(ii == 0), stop=(ii == IB - 1))
                a_sl = q2c_acc[0:1, ts(db, DBW)]
                if blk == 0:
                    nc.vector.tensor_copy(a_sl, qp)
                else:
                    nc.vector.tensor_add(a_sl, qp, a_sl)

            def m2_o2(blk=blk, E=E):
                # M2 (contract j) + O2 = alpha^T.T @ q * c, fused evacuation
                # into a full-width staging tile -> one 1MB DMA per row tile.
                for ii in range(IB):
                    ig = blk * IB + ii
                    o2f = outp.tile([P, Dd], F32, tag="o2f", name="o2f",
                                    bufs=CFG["outp_bufs"])
                    for db in range(DB):
                        u = ps_big.tile([P, DBW], F32, tag="m2", name="m2")
                        for j in range(JC):
                            nc.tensor.matmul(u, lhsT=E[j][:, ts(ii, P)],
                                             rhs=qb[j][:, ts(db, DBW)],
                                             start=(j == 0),
                                             stop=(j == JC - 1))
                        nc.vector.tensor_mul(o2f[:, ts(db, DBW)], u,
                                             cb[ig][:, ts(db, DBW)])
                    nc.sync.dma_start(out[b, ts(ig, P), Dd:2 * Dd], o2f)

            m2_o2()

        # ---- finalize beta/q2c ----
        bs = small.tile([P, 1], F32, tag="bsum", name="bsum")
        nc.vector.reduce_sum(bs, b_all[:, 0:IC], axis=AX.X)
        zb = ps_min.tile([1, 1], F32, tag="mini", name="mini", bufs=CFG.get("mini_bufs", 1))
        nc.tensor.matmul(zb, lhsT=ones_col_f, rhs=bs)
        zbr = small.tile([1, 1], F32, tag="zbr", name="zbr")
        nc.vector.reciprocal(zbr, zb)
        q2cb_row = rows.tile([1, Dd], BF16, tag="q2cbr", name="q2cbr")
        nc.vector.tensor_scalar(q2cb_row, q2c_acc, zbr, None, ALU.mult)
        q2c_bc = res.tile([P, Dd], BF16, tag="q2cbc", name="q2cbc")
        nc.gpsimd.partition_broadcast(q2c_bc, q2cb_row)
        # ---- O3 = c * q2c (broadcast over rows) -> one 1MB DMA per tile,
        # muls split DVE/GPSIMD and DMAs split ACT/Pool rings ----
        for ig in range(IC):
            o3f = outp.tile([P, Dd], F32, tag="o3f", name="o3f",
                            bufs=CFG["outp_bufs"])
            eng = nc.gpsimd if (ig % CFG["o3_dve_mod"] == 0) else nc.vector
            eng.tensor_mul(o3f, cb[ig], q2c_bc)
            dma_eng = nc.scalar if ig % 2 == 0 else nc.sync
            dma_eng.dma_start(out[b, ts(ig, P), 2 * Dd:3 * Dd], o3f)


def build(B_loc=B_LOC, QLd=QL, CLd=CL, Dd=D, rep=1):
    nc = bacc.Bacc("TRN2", target_bir_lowering=False, debug=False,
                   enable_asserts=False, num_devices=1)
    q_in = nc.dram_tensor("question", [B_loc, QLd, Dd], F32,
                          kind="ExternalInput").ap()
    c_in = nc.dram_tensor("context", [B_loc, CLd, Dd], F32,
                          kind="ExternalInput").ap()
    w_in = nc.dram_tensor("wsim", [3 * Dd], F32, kind="ExternalInput").ap()
    out = nc.dram_tensor("out", [B_loc, CLd, 3 * Dd], F32,
                         kind="ExternalOutput").ap()
    with tile.TileContext(nc) as tc, ExitStack() as ctx:
        emit(ctx, nc, tc, q_in, c_in, w_in, out, B_loc, QLd, CLd, Dd, rep=rep)
    nc.compile()
    return nc


_CACHED_NC = None


def _get_nc():
    global _CACHED_NC
    if _CACHED_NC is None:
        _CACHED_NC = build()
    return _CACHED_NC


def _shard(question, context, wsim):
    in_maps = []
    for i in range(N_CORES):
        in_maps.append({
            "question": np.ascontiguousarray(question[i * B_LOC:(i + 1) * B_LOC],
                                             dtype=np.float32),
            "context": np.ascontiguousarray(context[i * B_LOC:(i + 1) * B_LOC],
                                            dtype=np.float32),
            "wsim": np.ascontiguousarray(wsim, dtype=np.float32),
        })
    return in_maps


def kernel_raw(question, context, wsim, **run_kwargs):
    """Run and return the full BassKernelResults (for profiling)."""
    nc = _get_nc()
    in_maps = _shard(np.asarray(question), np.asarray(context),
                     np.asarray(wsim))
    res = run_bass_kernel_spmd(nc, in_maps, core_ids=list(range(N_CORES)),
                               **run_kwargs)
    return res


def kernel(question, context, wsim):
    res = kernel_raw(question, context, wsim)
    out = np.concatenate([res.results[i]["out"] for i in range(N_CORES)],
                         axis=0)
    return out.astype(np.float32)

